# revision 11
# baseline (speedup 1.0000x reference)
"""DispLoss Trainium2 kernel: 8-core SPMD Bass/Tile implementation.

reference semantics:
  protos = sequential-EMA-update(prototypes, features, labels)   # normalize each updated row
  logits = protos @ protos.T / 0.1
  loss   = mean(log((exp(logits).sum(1) - diag(exp(logits))) / (C-1)))

Sharding: EMA + transpose are sharded by prototype-row slice (core c owns rows
[c*1024, (c+1)*1024)); A^T shards are AllGathered; the CxC logits reduction is
row-sharded (core c computes rows of its slice); scalar partials AllReduced.
All per-core variation is carried by int32 index inputs (one SPMD program).
"""
import sys
import types
import math
import numpy as np

sys.path.insert(0, "/opt/trn_rl_repo")

N_CORES = 8
C = 8192          # n_class
D = 1024          # feat_dim
B = 1024          # batch
CPC = C // N_CORES  # rows per core
KCH = D // 128    # feature chunks
TEMP = 0.1
INV_TEMP = 1.0 / TEMP


def _shim_axon_hooks():
    try:
        from antenv.axon_hooks import get_axon_ntff_profile_hook  # noqa: F401
    except Exception:
        shim = types.ModuleType("antenv.axon_hooks")
        shim.get_axon_ntff_profile_hook = lambda: None
        sys.modules["antenv.axon_hooks"] = shim


def build_nc(round_tiles, loop_k=None):
    """Build the SPMD Bass program.

    round_tiles: list of tile counts per EMA round, e.g. [2, 1, 1]
                 (each tile processes 128 slots).
    loop_k: if set, wrap the compute phases in a For_i(0, loop_k) hardware
            loop (timing builds only; collectives stay outside the loop).
    """
    import concourse.mybir as mybir
    from concourse import bacc
    from concourse.tile import TileContext
    from concourse.bass import IndirectOffsetOnAxis
    from concourse.masks import make_identity
    from contextlib import ExitStack

    F32, BF16, I32 = mybir.dt.float32, mybir.dt.bfloat16, mybir.dt.int32
    AX = mybir.AxisListType
    OP = mybir.AluOpType
    AF = mybir.ActivationFunctionType

    nc = bacc.Bacc(None, target_bir_lowering=False, num_devices=N_CORES)

    feats = nc.declare_dram_parameter("features", [B, D], F32, isOutput=False)
    protos = nc.declare_dram_parameter("prototypes", [C, D], F32, isOutput=False)
    # index params are laid out [128, T]: partition p, tile t holds the index
    # for slot t*128+p (indirect-DMA offset vectors must be per-partition).
    base_idx = nc.declare_dram_parameter("base_idx", [128, CPC // 128], I32, isOutput=False)
    ridx = []
    for r, nt in enumerate(round_tiles):
        g = nc.declare_dram_parameter(f"g{r}", [128, nt], I32, isOutput=False)
        f = nc.declare_dram_parameter(f"f{r}", [128, nt], I32, isOutput=False)
        s = nc.declare_dram_parameter(f"s{r}", [128, nt], I32, isOutput=False)
        ridx.append((g, f, s))
    out = nc.declare_dram_parameter("out", [1, 1], F32, isOutput=True)

    with TileContext(nc) as tc:
        with ExitStack() as top:
            dram = top.enter_context(tc.tile_pool(name="dram", bufs=1, space="DRAM"))
            # P_loc row CPC is a trash row for padded scatter slots
            P_loc = dram.tile([CPC + 1, D], BF16)
            agin = dram.tile([KCH, 128, CPC], BF16)
            agout = dram.tile([N_CORES, KCH, 128, CPC], BF16, addr_space="Shared")
            ar_in = dram.tile([1, 1], F32)
            ar_out = dram.tile([1, 1], F32, addr_space="Shared")

            persist = top.enter_context(tc.tile_pool(name="persist", bufs=1))
            # SH: own transposed shard, [feat%128, feat//128, own col] bf16
            SH = persist.tile([128, KCH, CPC], BF16)
            # AT: full transposed prototypes, [feat%128, feat//128, class] bf16
            AT = persist.tile([128, KCH, C], BF16)
            ident = persist.tile([128, 128], BF16)
            make_identity(nc, ident[:])
            # per-row-block stats
            rowparts = persist.tile([128, 8, 8], F32)   # exp row-sums per (bi, e)
            dexp = persist.tile([128, 8], F32)          # diag exp per bi
            ones = persist.tile([128, 1], F32)
            nc.vector.memset(ones[:], 1.0)

            # index vectors
            idxp = top.enter_context(tc.tile_pool(name="idx", bufs=1))
            bidx_sb = idxp.tile([128, CPC // 128], I32)
            nc.sync.dma_start(out=bidx_sb[:], in_=base_idx[:, :])
            ridx_sb = []
            for r, (g, f, s) in enumerate(ridx):
                nt = round_tiles[r]
                gs = idxp.tile([128, nt], I32, name=f"g{r}sb")
                fs = idxp.tile([128, nt], I32, name=f"f{r}sb")
                ss = idxp.tile([128, nt], I32, name=f"s{r}sb")
                nc.sync.dma_start(out=gs[:], in_=g[:, :])
                nc.sync.dma_start(out=fs[:], in_=f[:, :])
                nc.sync.dma_start(out=ss[:], in_=s[:, :])
                ridx_sb.append((gs, fs, ss))

            def emit_ag():
                nc.gpsimd.collective_compute(
                    "AllGather", mybir.AluOpType.bypass,
                    replica_groups=[list(range(N_CORES))],
                    ins=[agin.opt()], outs=[agout.opt()],
                )

            def body(do_ag):
                # ---------------- Phase A: base-build P_loc + EMA ----------------
                with ExitStack() as ph:
                    sba = ph.enter_context(tc.tile_pool(name="phaseA", bufs=2))
                    for t in range(CPC // 128):
                        Lk = sba.tile([128, D], F32, tag="lk")
                        nc.gpsimd.indirect_dma_start(
                            out=Lk[:, :], out_offset=None,
                            in_=protos[:, :],
                            in_offset=IndirectOffsetOnAxis(
                                ap=bidx_sb[:, t:t + 1], axis=0),
                        )
                        Ck = sba.tile([128, D], BF16, tag="ck")
                        nc.scalar.copy(Ck[:], Lk[:])
                        nc.sync.dma_start(
                            out=P_loc[t * 128:(t + 1) * 128, :], in_=Ck[:])

                    for r in range(len(round_tiles)):
                        gs, fs, ss = ridx_sb[r]
                        for t in range(round_tiles[r]):
                            if r == 0:
                                G = sba.tile([128, D], F32, tag="gema")
                                nc.gpsimd.indirect_dma_start(
                                    out=G[:, :], out_offset=None,
                                    in_=protos[:, :],
                                    in_offset=IndirectOffsetOnAxis(ap=gs[:, t:t + 1], axis=0),
                                )
                            else:
                                G = sba.tile([128, D], BF16, tag="gema_bf")
                                nc.gpsimd.indirect_dma_start(
                                    out=G[:, :], out_offset=None,
                                    in_=P_loc[:, :],
                                    in_offset=IndirectOffsetOnAxis(ap=gs[:, t:t + 1], axis=0),
                                )
                            Ft = sba.tile([128, D], F32, tag="fema")
                            nc.gpsimd.indirect_dma_start(
                                out=Ft[:, :], out_offset=None,
                                in_=feats[:, :],
                                in_offset=IndirectOffsetOnAxis(ap=fs[:, t:t + 1], axis=0),
                            )
                            # m = f*(0.05/0.95) + g  (same direction as ref row
                            # up to a positive scale, which normalization removes)
                            M = sba.tile([128, D], F32, tag="mema")
                            nc.vector.scalar_tensor_tensor(
                                out=M[:], in0=Ft[:], scalar=0.05 / 0.95, in1=G[:],
                                op0=OP.mult, op1=OP.add,
                            )
                            # (tensor_tensor_reduce is broken on this HW path;
                            #  use mul + standard reduce instead)
                            Msq = sba.tile([128, D], F32, tag="msq")
                            nc.vector.tensor_mul(Msq[:], M[:], M[:])
                            ssq = sba.tile([128, 1], F32, tag="ssq")
                            nc.vector.reduce_sum(ssq[:], Msq[:], axis=AX.X)
                            rsq = sba.tile([128, 1], F32, tag="rsq")
                            nc.vector.reciprocal(rsq[:], ssq[:])
                            rnorm = sba.tile([128, 1], F32, tag="rnorm")
                            nc.scalar.activation(rnorm[:], rsq[:], AF.Sqrt)
                            Mn = sba.tile([128, D], BF16, tag="mn")
                            nc.vector.tensor_scalar_mul(Mn[:], M[:], rnorm[:])
                            nc.gpsimd.indirect_dma_start(
                                out=P_loc[:, :],
                                out_offset=IndirectOffsetOnAxis(ap=ss[:, t:t + 1], axis=0),
                                in_=Mn[:, :], in_offset=None,
                            )

                # ---------------- Phase B: transpose own slice -> SH -> agin ----
                with ExitStack() as ph:
                    sbb = ph.enter_context(tc.tile_pool(name="phaseB", bufs=3))
                    psb = ph.enter_context(tc.tile_pool(name="psB", bufs=4, space="PSUM"))
                    for R in range(CPC // 128):
                        T = sbb.tile([128, D], BF16, tag="trin")
                        nc.sync.dma_start(out=T[:], in_=P_loc[R * 128:(R + 1) * 128, :])
                        for k in range(KCH):
                            pt = psb.tile([128, 128], BF16, tag="pt")
                            nc.tensor.transpose(pt[:], T[:, k * 128:(k + 1) * 128], ident[:])
                            eng = nc.vector if (k % 2 == 0) else nc.scalar
                            if eng is nc.vector:
                                nc.vector.tensor_copy(SH[:, k, R * 128:(R + 1) * 128], pt[:])
                            else:
                                nc.scalar.copy(SH[:, k, R * 128:(R + 1) * 128], pt[:])
                    nc.sync.dma_start(
                        out=agin[:, :, :].rearrange("k p j -> p k j"), in_=SH[:, :, :])

                # --------------- Phase Dg: diag blocks from SH (pre-AT) ---------
                with ExitStack() as ph:
                    pdg = ph.enter_context(tc.tile_pool(name="psDiag", bufs=2, space="PSUM"))
                    sdg = ph.enter_context(tc.tile_pool(name="sbDiag", bufs=2))
                    for bi in range(8):
                        pd = pdg.tile([128, 128], F32, tag="pd")
                        for k in range(KCH):
                            nc.tensor.matmul(
                                pd[:],
                                SH[:, k, bi * 128:(bi + 1) * 128],
                                SH[:, k, bi * 128:(bi + 1) * 128],
                                start=(k == 0), stop=(k == KCH - 1),
                            )
                        ex = sdg.tile([128, 128], F32, tag="ex")
                        nc.scalar.activation(ex[:], pd[:], AF.Exp, scale=INV_TEMP)
                        dsel = sdg.tile([128, 128], F32, tag="dsel")
                        nc.gpsimd.affine_select(
                            out=dsel[:], in_=ex[:], pattern=[[1, 128]], base=0,
                            channel_multiplier=-1, compare_op=OP.is_equal, fill=0.0,
                        )
                        nc.vector.reduce_sum(dexp[:, bi:bi + 1], dsel[:], axis=AX.X)

                # ---------------- AllGather (non-loop builds) -------------------
                if do_ag:
                    emit_ag()

                # ---------------- Phase D: load AT from agout -------------------
                for rr in range(N_CORES):
                    for k in range(KCH):
                        nc.sync.dma_start(
                            out=AT[:, k, rr * CPC:(rr + 1) * CPC],
                            in_=agout[rr, k, :, :],
                        )

                # ---------------- Phase E: matmul + exp + rowsum ----------------
                with ExitStack() as ph:
                    pmm = ph.enter_context(tc.tile_pool(name="psMM", bufs=2, space="PSUM"))
                    scr = ph.enter_context(tc.tile_pool(name="scratch", bufs=2))
                    for bi in range(8):
                        for e in range(8):
                            ps = pmm.tile([128, 1024], F32, tag="ps")
                            for k in range(KCH):
                                lhsT = SH[:, k, bi * 128:(bi + 1) * 128]
                                for j in range(2):
                                    nc.tensor.matmul(
                                        ps[:, j * 512:(j + 1) * 512],
                                        lhsT,
                                        AT[:, k, e * 1024 + j * 512: e * 1024 + (j + 1) * 512],
                                        start=(k == 0), stop=(k == KCH - 1),
                                    )
                            sc = scr.tile([128, 1024], F32, tag="sc")
                            nc.scalar.activation(
                                sc[:], ps[:], AF.Exp, scale=INV_TEMP,
                                accum_out=rowparts[:, bi, e:e + 1],
                            )

                # ---------------- Phase F: local reduce -------------------------
                with ExitStack() as ph:
                    fin = ph.enter_context(tc.tile_pool(name="fin", bufs=1))
                    pfin = ph.enter_context(tc.tile_pool(name="psFin", bufs=1, space="PSUM"))
                    rs8 = fin.tile([128, 8], F32)
                    nc.vector.tensor_reduce(rs8[:], rowparts[:, :, :], axis=AX.X, op=OP.add)
                    sn = fin.tile([128, 8], F32)
                    nc.vector.tensor_sub(sn[:], rs8[:], dexp[:, :])
                    lg = fin.tile([128, 8], F32)
                    nc.scalar.activation(lg[:], sn[:], AF.Ln)
                    l1 = fin.tile([128, 1], F32)
                    nc.vector.reduce_sum(l1[:], lg[:], axis=AX.X)
                    pt1 = pfin.tile([1, 1], F32)
                    nc.tensor.matmul(pt1[:], ones[:], l1[:], start=True, stop=True)
                    tot = fin.tile([1, 1], F32)
                    nc.vector.tensor_copy(tot[:], pt1[:])
                    nc.sync.dma_start(out=ar_in[:, :], in_=tot[:])

            if loop_k is not None:
                # timing build: collectives cannot sit inside control flow, so
                # the loop body reads stale agout (zeros) — timing-equivalent.
                with tc.For_i(0, loop_k, 1):
                    body(do_ag=False)
                emit_ag()
            else:
                body(do_ag=True)

            # ---------------- AllReduce + final scale ---------------------
            nc.gpsimd.collective_compute(
                "AllReduce", mybir.AluOpType.add,
                replica_groups=[list(range(N_CORES))],
                ins=[ar_in.opt()], outs=[ar_out.opt()],
            )
            with ExitStack() as ph:
                fin2 = ph.enter_context(tc.tile_pool(name="fin2", bufs=1))
                ld = fin2.tile([1, 1], F32)
                nc.sync.dma_start(out=ld[:], in_=ar_out[:, :])
                fv = fin2.tile([1, 1], F32)
                # loss = total/C - log(C-1)
                nc.vector.tensor_scalar(
                    out=fv[:], in0=ld[:], scalar1=1.0 / C,
                    scalar2=-math.log(C - 1), op0=mybir.AluOpType.mult,
                    op1=mybir.AluOpType.add,
                )
                nc.sync.dma_start(out=out[:, :], in_=fv[:])

    nc.finalize()
    return nc


def host_prep(labels):
    """Compute per-core EMA round index arrays from labels (host-side int work).

    Returns (round_tiles, per_core_inputs) where per_core_inputs[c] is a dict
    of the int32 index arrays for core c.
    """
    labels = np.asarray(labels).astype(np.int64)
    Bn = labels.shape[0]
    occ = {}
    rnd = np.empty(Bn, dtype=np.int64)
    for i, l in enumerate(labels.tolist()):
        occ[l] = occ.get(l, 0) + 1
        rnd[i] = occ[l] - 1
    n_rounds = int(rnd.max()) + 1

    # slots per (round, core)
    slots = [[[] for _ in range(N_CORES)] for _ in range(n_rounds)]
    for i, l in enumerate(labels.tolist()):
        slots[rnd[i]][l // CPC].append(i)

    round_tiles = []
    for r in range(n_rounds):
        mx = max(len(slots[r][c]) for c in range(N_CORES))
        round_tiles.append(max(1, (mx + 127) // 128))

    per_core = []
    for c in range(N_CORES):
        # [128, T] layout: partition p, tile t = slot t*128+p
        base = np.arange(c * CPC, (c + 1) * CPC, dtype=np.int32)
        d = {"base_idx": np.ascontiguousarray(base.reshape(CPC // 128, 128).T)}
        for r in range(n_rounds):
            L = round_tiles[r] * 128
            g = np.zeros(L, dtype=np.int32)
            f = np.zeros(L, dtype=np.int32)
            s = np.full(L, CPC, dtype=np.int32)  # pad -> trash row
            for j, i in enumerate(slots[r][c]):
                l = int(labels[i])
                g[j] = l if r == 0 else (l - c * CPC)
                f[j] = i
                s[j] = l - c * CPC
            d[f"g{r}"] = np.ascontiguousarray(g.reshape(-1, 128).T)
            d[f"f{r}"] = np.ascontiguousarray(f.reshape(-1, 128).T)
            d[f"s{r}"] = np.ascontiguousarray(s.reshape(-1, 128).T)
        per_core.append(d)
    return round_tiles, per_core


_NC_CACHE = {}


def kernel(features, labels, prototypes):
    _shim_axon_hooks()
    from concourse.bass_utils import run_bass_kernel_spmd

    features = np.ascontiguousarray(np.asarray(features), dtype=np.float32)
    prototypes = np.ascontiguousarray(np.asarray(prototypes), dtype=np.float32)
    round_tiles, per_core = host_prep(labels)

    key = tuple(round_tiles)
    if key not in _NC_CACHE:
        _NC_CACHE[key] = build_nc(round_tiles)
    nc = _NC_CACHE[key]

    in_maps = []
    for c in range(N_CORES):
        m = {"features": features, "prototypes": prototypes}
        m.update(per_core[c])
        in_maps.append(m)

    res = run_bass_kernel_spmd(nc, in_maps, core_ids=list(range(N_CORES)), trace=False)
    val = np.float32(res.results[0]["out"][0, 0])
    return np.asarray(val, dtype=np.float32).reshape(())
